# revision 2
# baseline (speedup 1.0000x reference)
"""Gaussian-kernel attention for Trainium2 (Bass/Tile), 8-core data-parallel.

Computes out = x + K @ x with K = exp(-r * d2), d2[t,s] = ||x_t - x_s||^2,
per batch.  Decomposition used on-chip:

    d2 = sq_t + sq_s - 2*G          (G = X X^T, sq = rowwise |x|^2)
    K  = e_t * S * e_s              (S = exp(2r*G), e_i = exp(-r*sq_i))
    out[u] = x[u] + e_u * Z[u],  Z[u] = sum_v S[u,v] * (e_v x[v])

S is symmetric, which lets mm2 contract over the PARTITION dim of the
S-stripes directly (Z^T = sum_i Y_i^T S_i with S_i = stripe [v-block i,
all u]) -- no per-tile transposes anywhere in the main loop.

Per batch the main loop runs over 32 half-stripes (16 t-blocks x 2
s-halves of 1024).  PSUM budget (8 banks):
  gps [128, 6, 512]  -- 3 rotation slots of 2 banks for G half-stripes
  pz  [128, 2, 512]  -- Z^T col-packed: parts 0-63 = u 0:1024,
                        parts 64-127 = u 1024:2048 (4 accumulation
                        chains, verified safe: has_written clear is
                        per-partition)
PE packing: mm1 (K=64) alternates row groups 0/64 per stripe so
consecutive stripes' matmuls run concurrently; mm2 (M=64) alternates
col groups 0/64 per s-half.  ACT (exp) is the roofline engine:
32 x N=1024 instructions per batch.

Sharding: pure data-parallel over batch B=32 -> 4 batches per core x 8.
"""

import os
import sys

import numpy as np

sys.path.insert(0, "/opt/trn_rl_repo")

import concourse.bass as bass
import concourse.tile as tile
from concourse import bacc, mybir
from concourse.bass_utils import run_bass_kernel_spmd

FP32 = mybir.dt.float32
BF16 = mybir.dt.bfloat16

B, T, C = 32, 2048, 64
N_CORES = 8
BPC = B // N_CORES  # batches per core

# Stashed by kernel() for the test harness (exec time etc.)
LAST_RESULTS = None


def _body(ctx, tc, out_ap, x_ap, r, bpc, t, dbg=False):
    """Emit the per-core kernel IR.

    out_ap/x_ap: DRAM APs of shape [bpc, t, C].
    r: python float (r_sigma value, baked as immediates).
    """
    nc = tc.nc

    def dump(name, sb_ap, dt=None):
        if not dbg:
            return
        d = nc.dram_tensor(
            name, list(sb_ap.shape), dt or sb_ap.dtype, kind="ExternalOutput"
        ).ap()
        nc.sync.dma_start(out=d, in_=sb_ap)

    nt = t // 128          # t-blocks (stripes) per batch
    nh = 2 * nt            # half-stripes per batch (1024 s-cols each)
    exp2r = 2.0 * r

    # SBUF pools (bufs=2 for cross-batch pipelining)
    xpool = ctx.enter_context(tc.tile_pool(name="x32", bufs=2))
    xxpool = ctx.enter_context(tc.tile_pool(name="xx", bufs=2))
    sqpool = ctx.enter_context(tc.tile_pool(name="sq", bufs=2))
    ypool = ctx.enter_context(tc.tile_pool(name="yb", bufs=2))
    xbpool = ctx.enter_context(tc.tile_pool(name="xbp", bufs=2))
    xtpool = ctx.enter_context(tc.tile_pool(name="xt", bufs=2))
    apool = ctx.enter_context(tc.tile_pool(name="a0", bufs=4))
    opool = ctx.enter_context(tc.tile_pool(name="osb", bufs=2))
    # PSUM: 3-slot G rotation (6 banks) + col-packed Z^T accum (2 banks)
    gpool = ctx.enter_context(tc.tile_pool(name="gps", bufs=1, space="PSUM"))
    ppool = ctx.enter_context(tc.tile_pool(name="pps", bufs=1, space="PSUM"))
    # DRAM scratch for the bf16 transpose round-trip
    dpool = ctx.enter_context(tc.tile_pool(name="dsc", bufs=2, space="DRAM"))

    gps = gpool.tile([128, 6, 512], FP32)   # banks 0-5, rotation slots
    pz = ppool.tile([128, 2, 512], FP32)    # banks 6-7, Z^T col-packed

    for b in range(bpc):
        xb_dram = x_ap[b].rearrange("(k p) c -> p k c", p=128)   # [128, nt, C]
        ob_dram = out_ap[b].rearrange("(k p) c -> p k c", p=128)

        # ---- prologue: load, row stats, Y = e_v * x (bf16), X^T (bf16) ----
        x32 = xpool.tile([128, nt, C], FP32)
        nc.sync.dma_start(out=x32[:], in_=xb_dram)

        xx = xxpool.tile([128, nt, C], FP32)
        nc.vector.tensor_mul(xx[:], x32[:], x32[:])
        sq = sqpool.tile([128, nt], FP32, tag="sq")
        nc.vector.tensor_reduce(
            sq[:], xx[:], axis=mybir.AxisListType.X, op=mybir.AluOpType.add
        )
        ev = sqpool.tile([128, nt], FP32, tag="ev")
        nc.scalar.activation(
            ev[:], sq[:], mybir.ActivationFunctionType.Exp, scale=-r
        )

        yb = ypool.tile([128, nt, C], BF16)
        for k in range(nt):
            nc.vector.tensor_scalar_mul(yb[:, k], x32[:, k], ev[:, k : k + 1])

        # bf16 copy of x written TWICE side by side (cols 0:C and C:2C) so a
        # single full-width DMA-xbar transpose yields X^T duplicated on both
        # partition halves -- lets mm1 alternate PE row groups per stripe.
        xbp = xbpool.tile([128, nt, 2 * C], BF16)
        nc.vector.tensor_copy(xbp[:, :, 0:C], x32[:])
        nc.vector.tensor_copy(xbp[:, :, C : 2 * C], x32[:])
        xbd = dpool.tile([t, 2 * C], BF16)
        nc.sync.dma_start(out=xbd.rearrange("(k p) c -> p k c", p=128), in_=xbp[:])
        xt = xtpool.tile([128, t], BF16)
        nc.sync.dma_start_transpose(out=xt[:], in_=xbd[:])
        # xt[c, tt] = xt[64+c, tt] = x[tt, c] for c < 64.
        if dbg and b == 0:
            dump("dbg_sq", sq[:])
            dump("dbg_ev", ev[:])
            dump("dbg_yb", yb[:])
            dump("dbg_xt", xt[:])

        # ---- main loop: 32 half-stripes, software-pipelined emission ----
        # step m emits: mm1(m) -> ACT(m-1) -> mm2(m-2) so the PE queue
        # never has a blocked mm2 at its head starving ready mm1 work.
        a0_tiles = [None] * nh

        def mm1(m):
            i, h = m // 2, m % 2
            s = m % 3
            rows = 64 * (i % 2)
            for n in range(2):
                nc.tensor.matmul(
                    gps[:, 2 * s + n],
                    lhsT=xt[rows : rows + 64, i * 128 : (i + 1) * 128],
                    rhs=xt[
                        rows : rows + 64,
                        h * 1024 + n * 512 : h * 1024 + (n + 1) * 512,
                    ],
                    start=True,
                    stop=True,
                    skip_group_check=True,
                )

        def act(m):
            s = m % 3
            a0 = apool.tile([128, 2, 512], BF16)
            nc.scalar.activation(
                a0[:],
                gps[:, 2 * s : 2 * s + 2],
                mybir.ActivationFunctionType.Exp,
                scale=exp2r,
            )
            a0_tiles[m] = a0
            if dbg and b == 0 and m == 0:
                gsb = xxpool.tile([128, 2, 512], FP32, tag="gdump")
                nc.vector.tensor_copy(gsb[:], gps[:, 0:2])
                dump("dbg_g00", gsb[:])
                dump("dbg_a00", a0[:])

        def mm2(m):
            i, h = m // 2, m % 2
            a0 = a0_tiles[m]
            a0_tiles[m] = None
            for n in range(2):
                # chain (h, n): parts 64h..64h+64, bank 6+n; start/stop per i
                nc.tensor.matmul(
                    pz[64 * h : 64 * h + 64, n],
                    lhsT=yb[:, i],
                    rhs=a0[:, n],
                    start=(i == 0),
                    stop=(i == nt - 1),
                    skip_group_check=True,
                )

        for m in range(nh + 2):
            if m < nh:
                mm1(m)
            if 1 <= m < nh + 1:
                act(m - 1)
            if m >= 2:
                mm2(m - 2)

        # ---- epilogue: Z^T -> bf16 -> DMA-xbar transpose -> out ----
        zt = opool.tile([128, 2, 512], BF16, tag="zt")
        nc.vector.tensor_copy(zt[:], pz[:])
        tr = opool.tile([128, nt, C], BF16, tag="tr")
        for j in range(nt):
            h, n, off = j // 8, (j // 4) % 2, (j % 4) * 128
            nc.sync.dma_start_transpose(
                out=tr[:, j], in_=zt[64 * h : 64 * h + 64, n, off : off + 128]
            )
        osb = opool.tile([128, nt, C], FP32, tag="osb")
        for j in range(nt):
            nc.vector.scalar_tensor_tensor(
                osb[:, j],
                in0=tr[:, j],
                scalar=ev[:, j : j + 1],
                in1=x32[:, j],
                op0=mybir.AluOpType.mult,
                op1=mybir.AluOpType.add,
            )
        nc.sync.dma_start(out=ob_dram, in_=osb[:])


def build(r, bpc=BPC, t=T, dbg=False):
    """Build + compile the Bass module for one core's shard."""
    from contextlib import ExitStack

    nc = bacc.Bacc(
        "TRN2", target_bir_lowering=False, debug=False, num_devices=N_CORES
    )
    x_ap = nc.dram_tensor("x", [bpc, t, C], FP32, kind="ExternalInput").ap()
    out_ap = nc.dram_tensor("out", [bpc, t, C], FP32, kind="ExternalOutput").ap()
    with tile.TileContext(nc) as tc:
        with ExitStack() as ctx:
            _body(ctx, tc, out_ap, x_ap, r, bpc, t, dbg=dbg)
    nc.compile()
    return nc


def kernel(x, r_sigma):
    global LAST_RESULTS
    x = np.ascontiguousarray(np.asarray(x, dtype=np.float32))
    r = float(np.asarray(r_sigma).reshape(-1)[0])
    assert x.shape == (B, T, C), x.shape

    nc = build(r)
    in_maps = [
        {"x": np.ascontiguousarray(x[i * BPC : (i + 1) * BPC])}
        for i in range(N_CORES)
    ]
    trace = bool(int(os.environ.get("KERNEL_TRACE", "0")))
    res = run_bass_kernel_spmd(
        nc, in_maps, core_ids=list(range(N_CORES)), trace=trace
    )
    LAST_RESULTS = res
    out = np.concatenate([res.results[i]["out"] for i in range(N_CORES)], axis=0)
    return out.astype(np.float32)


# revision 5
# speedup vs baseline: 2.0502x; 2.0502x over previous
"""Gaussian-kernel attention for Trainium2 (Bass/Tile), 8-core data-parallel.

Computes out = x + K @ x with K = exp(-r * d2), d2[t,s] = ||x_t - x_s||^2,
per batch.  Decomposition used on-chip:

    d2 = sq_t + sq_s - 2*G          (G = X X^T, sq = rowwise |x|^2)
    K  = e_t * S * e_s              (S = exp(2r*G), e_i = exp(-r*sq_i))
    out[u] = x[u] + e_u * Z[u],  Z[u] = sum_v S[u,v] * (e_v x[v])

S is symmetric, which lets mm2 contract over the PARTITION dim of the
S-stripes directly (Z^T = sum_i Y_i^T S_i with S_i = stripe [v-block i,
all u]) -- no per-tile transposes anywhere in the main loop.

Per batch the main loop runs over 32 half-stripes (16 t-blocks x 2
s-halves of 1024).  PSUM budget (8 banks):
  gps [128, 6, 512]  -- 3 rotation slots of 2 banks for G half-stripes
  pz  [128, 2, 512]  -- Z^T col-packed: parts 0-63 = u 0:1024,
                        parts 64-127 = u 1024:2048 (4 accumulation
                        chains, verified safe: has_written clear is
                        per-partition)
PE packing: mm1 (K=64) alternates row groups 0/64 per stripe so
consecutive stripes' matmuls run concurrently; mm2 (M=64) alternates
col groups 0/64 per s-half.  ACT (exp) is the roofline engine:
32 x N=1024 instructions per batch.

Sharding: pure data-parallel over batch B=32 -> 4 batches per core x 8.
"""

import os
import sys

import numpy as np

sys.path.insert(0, "/opt/trn_rl_repo")

import concourse.bass as bass
import concourse.tile as tile
from concourse import bacc, mybir
from concourse.bass_utils import run_bass_kernel_spmd

FP32 = mybir.dt.float32
BF16 = mybir.dt.bfloat16

B, T, C = 32, 2048, 64
N_CORES = 8
BPC = B // N_CORES  # batches per core

# Stashed by kernel() for the test harness (exec time etc.)
LAST_RESULTS = None


def _body(ctx, tc, out_ap, x_ap, r, bpc, t, dbg=False):
    """Emit the per-core kernel IR.

    out_ap/x_ap: DRAM APs of shape [bpc, t, C].
    r: python float (r_sigma value, baked as immediates).
    """
    nc = tc.nc

    def dump(name, sb_ap, dt=None):
        if not dbg:
            return
        d = nc.dram_tensor(
            name, list(sb_ap.shape), dt or sb_ap.dtype, kind="ExternalOutput"
        ).ap()
        nc.sync.dma_start(out=d, in_=sb_ap)

    nt = t // 128          # t-blocks (stripes) per batch
    nh = 2 * nt            # half-stripes per batch (1024 s-cols each)
    exp2r = 2.0 * r

    # SBUF pools (bufs=2 for cross-batch pipelining)
    xpool = ctx.enter_context(tc.tile_pool(name="x32", bufs=2))
    xxpool = ctx.enter_context(tc.tile_pool(name="xx", bufs=2))
    sqpool = ctx.enter_context(tc.tile_pool(name="sq", bufs=2))
    ypool = ctx.enter_context(tc.tile_pool(name="yb", bufs=2))
    xbpool = ctx.enter_context(tc.tile_pool(name="xbp", bufs=2))
    xtpool = ctx.enter_context(tc.tile_pool(name="xt", bufs=2))
    apool = ctx.enter_context(tc.tile_pool(name="a0", bufs=4))
    opool = ctx.enter_context(tc.tile_pool(name="osb", bufs=2))
    # PSUM: 3-slot G rotation (6 banks) + col-packed Z^T accum (2 banks).
    # One tile PER SLOT (bufs=3): Tile tracks deps per tile object, so the
    # WAR wait for refilling a slot lands on the ACT that read THAT slot
    # (lag 3), not the latest ACT (lag 1 -> lockstep).
    gpool = ctx.enter_context(tc.tile_pool(name="gps", bufs=3, space="PSUM"))
    ppool = ctx.enter_context(tc.tile_pool(name="pps", bufs=1, space="PSUM"))
    # DRAM scratch for the bf16 transpose round-trip
    dpool = ctx.enter_context(tc.tile_pool(name="dsc", bufs=2, space="DRAM"))

    pz = ppool.tile([128, 2, 512], FP32)    # 2 banks, Z^T col-packed

    for b in range(bpc):
        xb_dram = x_ap[b].rearrange("(k p) c -> p k c", p=128)   # [128, nt, C]
        ob_dram = out_ap[b].rearrange("(k p) c -> p k c", p=128)

        # ---- prologue ----
        # Critical path first: x -> bf16 dup -> DRAM round-trip -> X^T, so
        # mm1 can start ASAP.  Row stats (sq/ev/yb, only needed by mm2 and
        # the epilogue) are emitted after and overlap the DMA round-trip.
        x32 = xpool.tile([128, nt, C], FP32)
        nc.sync.dma_start(out=x32[:], in_=xb_dram)

        # bf16 copy of x written TWICE side by side (cols 0:C and C:2C) so a
        # single full-width DMA-xbar transpose yields X^T duplicated on both
        # partition halves -- lets mm1 alternate PE row groups per stripe.
        xbp = xbpool.tile([128, nt, 2 * C], BF16)
        nc.vector.tensor_copy(xbp[:, :, 0:C], x32[:])
        nc.vector.tensor_copy(xbp[:, :, C : 2 * C], x32[:])
        xbd = dpool.tile([t, 2 * C], BF16)
        nc.sync.dma_start(out=xbd.rearrange("(k p) c -> p k c", p=128), in_=xbp[:])
        xt = xtpool.tile([128, t], BF16)
        nc.sync.dma_start_transpose(out=xt[:], in_=xbd[:])
        # xt[c, tt] = xt[64+c, tt] = x[tt, c] for c < 64.

        xx = xxpool.tile([128, nt, C], FP32)
        nc.vector.tensor_mul(xx[:], x32[:], x32[:])
        sq = sqpool.tile([128, nt], FP32, tag="sq")
        nc.vector.tensor_reduce(
            sq[:], xx[:], axis=mybir.AxisListType.X, op=mybir.AluOpType.add
        )
        ev = sqpool.tile([128, nt], FP32, tag="ev")
        nc.scalar.activation(
            ev[:], sq[:], mybir.ActivationFunctionType.Exp, scale=-r
        )

        yb = ypool.tile([128, nt, C], BF16)
        for k in range(nt):
            nc.vector.tensor_scalar_mul(yb[:, k], x32[:, k], ev[:, k : k + 1])
        if dbg and b == 0:
            dump("dbg_sq", sq[:])
            dump("dbg_ev", ev[:])
            dump("dbg_yb", yb[:])
            dump("dbg_xt", xt[:])

        # ---- main loop: 32 half-stripes, software-pipelined emission ----
        # step m emits: mm2(m-2) -> mm1(m) -> ACT(m-1).  mm2's dependency
        # (ACT two steps back) is the oldest, so the PE queue head never
        # blocks on fresh work.
        a0_tiles = [None] * nh
        g_tiles = [None] * nh

        def mm1(m):
            i, h = m // 2, m % 2
            rows = 64 * (i % 2)
            g = gpool.tile([128, 2, 512], FP32)
            g_tiles[m] = g
            for n in range(2):
                nc.tensor.matmul(
                    g[:, n],
                    lhsT=xt[rows : rows + 64, i * 128 : (i + 1) * 128],
                    rhs=xt[
                        rows : rows + 64,
                        h * 1024 + n * 512 : h * 1024 + (n + 1) * 512,
                    ],
                    start=True,
                    stop=True,
                    skip_group_check=True,
                )

        def act(m):
            g = g_tiles[m]
            g_tiles[m] = None
            a0 = apool.tile([128, 2, 512], BF16)
            nc.scalar.activation(
                a0[:],
                g[:],
                mybir.ActivationFunctionType.Exp,
                scale=exp2r,
            )
            a0_tiles[m] = a0
            if dbg and b == 0 and m == 0:
                gsb = xxpool.tile([128, 2, 512], FP32, tag="gdump")
                nc.vector.tensor_copy(gsb[:], g[:])
                dump("dbg_g00", gsb[:])
                dump("dbg_a00", a0[:])

        def mm2(m):
            i, h = m // 2, m % 2
            a0 = a0_tiles[m]
            a0_tiles[m] = None
            for n in range(2):
                # chain (h, n): parts 64h..64h+64, bank 6+n; start/stop per i
                nc.tensor.matmul(
                    pz[64 * h : 64 * h + 64, n],
                    lhsT=yb[:, i],
                    rhs=a0[:, n],
                    start=(i == 0),
                    stop=(i == nt - 1),
                    skip_group_check=True,
                )

        for m in range(nh + 2):
            if m >= 2:
                mm2(m - 2)
            if m < nh:
                mm1(m)
            if 1 <= m < nh + 1:
                act(m - 1)

        # ---- epilogue: Z^T -> bf16 -> DMA-xbar transpose -> out ----
        zt = opool.tile([128, 2, 512], BF16, tag="zt")
        nc.vector.tensor_copy(zt[:], pz[:])
        # one transpose per partition half: [64, 1024] -> [128, 8, C]
        # tr[p, j, c] = zt[64h+c, j*128+p] = Z[u, c] for u-block 8h+j
        tr = opool.tile([128, 2, 8, C], BF16, tag="tr")
        for h in range(2):
            nc.sync.dma_start_transpose(
                out=tr[:, h], in_=zt[64 * h : 64 * h + 64, :, :]
            )
        trv = tr[:].rearrange("p h j c -> p (h j) c")
        osb = opool.tile([128, nt, C], FP32, tag="osb")
        for j in range(nt):
            nc.vector.scalar_tensor_tensor(
                osb[:, j],
                in0=trv[:, j],
                scalar=ev[:, j : j + 1],
                in1=x32[:, j],
                op0=mybir.AluOpType.mult,
                op1=mybir.AluOpType.add,
            )
        nc.sync.dma_start(out=ob_dram, in_=osb[:])


def build(r, bpc=BPC, t=T, dbg=False):
    """Build + compile the Bass module for one core's shard."""
    from contextlib import ExitStack

    nc = bacc.Bacc(
        "TRN2", target_bir_lowering=False, debug=False, num_devices=N_CORES
    )
    x_ap = nc.dram_tensor("x", [bpc, t, C], FP32, kind="ExternalInput").ap()
    out_ap = nc.dram_tensor("out", [bpc, t, C], FP32, kind="ExternalOutput").ap()
    with tile.TileContext(nc) as tc:
        with ExitStack() as ctx:
            _body(ctx, tc, out_ap, x_ap, r, bpc, t, dbg=dbg)
    nc.compile()
    return nc


def kernel(x, r_sigma):
    global LAST_RESULTS
    x = np.ascontiguousarray(np.asarray(x, dtype=np.float32))
    r = float(np.asarray(r_sigma).reshape(-1)[0])
    assert x.shape == (B, T, C), x.shape

    nc = build(r)
    in_maps = [
        {"x": np.ascontiguousarray(x[i * BPC : (i + 1) * BPC])}
        for i in range(N_CORES)
    ]
    trace = bool(int(os.environ.get("KERNEL_TRACE", "0")))
    res = run_bass_kernel_spmd(
        nc, in_maps, core_ids=list(range(N_CORES)), trace=trace
    )
    LAST_RESULTS = res
    out = np.concatenate([res.results[i]["out"] for i in range(N_CORES)], axis=0)
    return out.astype(np.float32)


# revision 7
# speedup vs baseline: 2.1507x; 1.0490x over previous
"""Gaussian-kernel attention for Trainium2 (Bass/Tile), 8-core data-parallel.

Computes out = x + K @ x with K = exp(-r * d2), d2[t,s] = ||x_t - x_s||^2,
per batch.  Decomposition used on-chip:

    d2 = sq_t + sq_s - 2*G          (G = X X^T, sq = rowwise |x|^2)
    K  = e_t * S * e_s              (S = exp(2r*G), e_i = exp(-r*sq_i))
    out[u] = x[u] + e_u * Z[u],  Z[u] = sum_v S[u,v] * (e_v x[v])

S is symmetric, which lets mm2 contract over the PARTITION dim of the
S-stripes directly (Z^T = sum_i Y_i^T S_i with S_i = stripe [v-block i,
all u]) -- no per-tile transposes anywhere in the main loop.

The work is streamed as 64 chunks per batch (chunk = [128 t, 512 s] of
G/S), grouped into PSUM tiles of 3 chunks so each exp ACTIVATE covers
N=1536 (ACT is the roofline engine; bigger N amortizes its ~260ns
per-instruction overhead).  Chunks are ordered in two passes over the
s/u quarters -- (q=0,1 for all stripes) then (q=2,3) -- so the first
output half's epilogue runs mid-loop, hiding the tail, and the first
tiles only need the first X^T half (shorter prologue latency).

PSUM budget (8 banks):
  g tiles [128, 3, 512] x2 bufs -- 6 banks, ping-pong
  pz      [128, 2, 512]         -- 2 banks, Z^T col-packed: 4 interleaved
          accumulation chains keyed by the s/u quarter (has_written clear
          is per-partition, verified on HW, so chains may share banks)
PE packing: mm1 (K=64) alternates row groups 0/64 per stripe; mm2 (M=64)
uses col groups 0/64 per output half.

Sharding: pure data-parallel over batch B=32 -> 4 batches per core x 8.
"""

import os
import sys

import numpy as np

sys.path.insert(0, "/opt/trn_rl_repo")

import concourse.bass as bass
import concourse.tile as tile
from concourse import bacc, mybir
from concourse.bass_utils import run_bass_kernel_spmd

FP32 = mybir.dt.float32
BF16 = mybir.dt.bfloat16

B, T, C = 32, 2048, 64
N_CORES = 8
BPC = B // N_CORES  # batches per core

# Stashed by kernel() for the test harness (exec time etc.)
LAST_RESULTS = None


def _body(ctx, tc, out_ap, x_ap, r, bpc, t, dbg=False):
    """Emit the per-core kernel IR.

    out_ap/x_ap: DRAM APs of shape [bpc, t, C].
    r: python float (r_sigma value, baked as immediates).
    """
    nc = tc.nc

    def dump(name, sb_ap, dt=None):
        if not dbg:
            return
        d = nc.dram_tensor(
            name, list(sb_ap.shape), dt or sb_ap.dtype, kind="ExternalOutput"
        ).ap()
        nc.sync.dma_start(out=d, in_=sb_ap)

    nt = t // 128          # t-blocks (stripes) per batch
    hh = nt // 2
    nck = 4 * nt           # 512-wide chunks per batch
    ntile = (nck + 2) // 3  # 3-chunk pipeline tiles per batch
    exp2r = 2.0 * r

    # chunk stream: pass 0 = quarters (0, 1) for all stripes, pass 1 = (2, 3)
    chunks = [(i, q) for qp in range(2) for i in range(nt) for q in (2 * qp, 2 * qp + 1)]
    pass0_last_tile = (2 * nt - 1) // 3

    # SBUF pools (2 slots per tag for cross-batch pipelining)
    xpool = ctx.enter_context(tc.tile_pool(name="x32", bufs=2))
    xxpool = ctx.enter_context(tc.tile_pool(name="xx", bufs=2))
    sqpool = ctx.enter_context(tc.tile_pool(name="sq", bufs=2))
    ypool = ctx.enter_context(tc.tile_pool(name="yb", bufs=2))
    xbpool = ctx.enter_context(tc.tile_pool(name="xbp", bufs=2))
    xtpool = ctx.enter_context(tc.tile_pool(name="xt", bufs=2))
    apool = ctx.enter_context(tc.tile_pool(name="a0", bufs=5))
    opool = ctx.enter_context(tc.tile_pool(name="osb", bufs=2))
    # PSUM: 2 ping-pong G tiles of 3 banks + col-packed Z^T accum (2 banks)
    gpool = ctx.enter_context(tc.tile_pool(name="gps", bufs=2, space="PSUM"))
    ppool = ctx.enter_context(tc.tile_pool(name="pps", bufs=1, space="PSUM"))
    # DRAM scratch for the bf16 transpose round-trip
    dpool = ctx.enter_context(tc.tile_pool(name="dsc", bufs=2, space="DRAM"))

    pz = ppool.tile([128, 2, 512], FP32)    # 2 banks, Z^T col-packed

    for b in range(bpc):
        xb_dram = x_ap[b].rearrange("(k p) c -> p k c", p=128)   # [128, nt, C]
        ob_dram = out_ap[b].rearrange("(k p) c -> p k c", p=128)

        # ---- prologue, chunked in two t-halves so mm1 can start after the
        # first half's x -> bf16-dup -> DRAM round-trip -> X^T chain ----
        x32 = []   # two [128, hh, C] tiles
        xth = []   # two [128, 1024] X^T tiles (dup rows 0:64 / 64:128)
        for u in range(2):
            xh = xpool.tile([128, hh, C], FP32, tag=f"x32{u}")
            nc.sync.dma_start(out=xh[:], in_=xb_dram[:, u * hh : (u + 1) * hh])
            x32.append(xh)
            xbp = xbpool.tile([128, hh, 2 * C], BF16, tag=f"xbp{u}")
            nc.vector.tensor_copy(xbp[:, :, 0:C], xh[:])
            nc.vector.tensor_copy(xbp[:, :, C : 2 * C], xh[:])
            xbd = dpool.tile([t // 2, 2 * C], BF16, tag=f"xbd{u}")
            nc.sync.dma_start(
                out=xbd.rearrange("(k p) c -> p k c", p=128), in_=xbp[:]
            )
            xt = xtpool.tile([128, t // 2], BF16, tag=f"xt{u}")
            nc.sync.dma_start_transpose(out=xt[:], in_=xbd[:])
            xth.append(xt)
            # xt[c, tt] = xt[64+c, tt] = x[u*1024 + tt, c] for c < 64.

        # row stats: sq/ev (for the epilogue), yb = e_v * x (for mm2)
        sq = sqpool.tile([128, nt], FP32, tag="sq")
        for u in range(2):
            xx = xxpool.tile([128, hh, C], FP32, tag=f"xx{u}")
            nc.vector.tensor_mul(xx[:], x32[u][:], x32[u][:])
            nc.vector.tensor_reduce(
                sq[:, u * hh : (u + 1) * hh],
                xx[:],
                axis=mybir.AxisListType.X,
                op=mybir.AluOpType.add,
            )
        ev = sqpool.tile([128, nt], FP32, tag="ev")
        nc.scalar.activation(
            ev[:], sq[:], mybir.ActivationFunctionType.Exp, scale=-r
        )
        yb = ypool.tile([128, nt, C], BF16)
        for k in range(nt):
            nc.vector.tensor_scalar_mul(
                yb[:, k], x32[k // hh][:, k % hh], ev[:, k : k + 1]
            )

        if dbg and b == 0:
            dump("dbg_sq", sq[:])
            dump("dbg_ev", ev[:])
            dump("dbg_yb", yb[:])
            dump("dbg_xt", xth[0][:])

        # ---- main loop over 3-chunk tiles, software-pipelined emission:
        # step k emits mm2(k-2) -> mm1(k) -> ACT(k-1) so the PE queue head
        # never blocks on fresh work. ----
        a0_tiles = [None] * ntile
        g_tiles = [None] * ntile

        def nch(k):
            return min(3, nck - 3 * k)

        def mm1(k):
            g = gpool.tile([128, 3, 512], FP32)
            g_tiles[k] = g
            for j in range(nch(k)):
                i, q = chunks[3 * k + j]
                rows = 64 * (i % 2)
                xl = xth[i // hh]         # lhsT t-block half
                xr = xth[q // 2]          # rhs s-cols half
                nc.tensor.matmul(
                    g[:, j],
                    lhsT=xl[rows : rows + 64, (i % hh) * 128 : (i % hh + 1) * 128],
                    rhs=xr[rows : rows + 64, (q % 2) * 512 : (q % 2 + 1) * 512],
                    start=True,
                    stop=True,
                    skip_group_check=True,
                )

        def act(k):
            n = nch(k)
            g = g_tiles[k]
            g_tiles[k] = None
            a0 = apool.tile([128, 3, 512], BF16)
            nc.scalar.activation(
                a0[:, 0:n],
                g[:, 0:n],
                mybir.ActivationFunctionType.Exp,
                scale=exp2r,
            )
            a0_tiles[k] = a0
            if dbg and b == 0 and k == 0:
                gsb = xxpool.tile([128, 3, 512], FP32, tag="gdump")
                nc.vector.tensor_copy(gsb[:], g[:])
                dump("dbg_g00", gsb[:])
                dump("dbg_a00", a0[:])

        def mm2(k):
            a0 = a0_tiles[k]
            a0_tiles[k] = None
            for j in range(nch(k)):
                i, q = chunks[3 * k + j]
                # chain q: parts 64*(q//2).., bank q%2; start/stop per stripe
                nc.tensor.matmul(
                    pz[64 * (q // 2) : 64 * (q // 2) + 64, q % 2],
                    lhsT=yb[:, i],
                    rhs=a0[:, j],
                    start=(i == 0),
                    stop=(i == nt - 1),
                    skip_group_check=True,
                )

        # ---- epilogue (one output half): Z^T -> bf16 -> DMA-xbar
        # transpose -> e_u * Z + x -> DRAM.
        # tr[p, j, c] = zt[64h+c, j*128+p] = Z[u, c] for u-block 8h+j
        def epilogue(h):
            zt = opool.tile([128, 2, 512], BF16, tag=f"zt{h}")
            nc.vector.tensor_copy(
                zt[64 * h : 64 * h + 64, :, :], pz[64 * h : 64 * h + 64, :, :]
            )
            tr = opool.tile([128, hh, C], BF16, tag=f"tr{h}")
            nc.sync.dma_start_transpose(
                out=tr[:], in_=zt[64 * h : 64 * h + 64, :, :]
            )
            osb = opool.tile([128, hh, C], FP32, tag=f"osb{h}")
            for jj in range(hh):
                j = h * hh + jj
                nc.vector.scalar_tensor_tensor(
                    osb[:, jj],
                    in0=tr[:, jj],
                    scalar=ev[:, j : j + 1],
                    in1=x32[j // hh][:, j % hh],
                    op0=mybir.AluOpType.mult,
                    op1=mybir.AluOpType.add,
                )
            nc.sync.dma_start(
                out=ob_dram[:, h * hh : (h + 1) * hh], in_=osb[:]
            )

        for k in range(ntile + 2):
            if k >= 2:
                mm2(k - 2)
                if k - 2 == pass0_last_tile:
                    epilogue(0)
            if k < ntile:
                mm1(k)
            if 1 <= k < ntile + 1:
                act(k - 1)
        epilogue(1)


def build(r, bpc=BPC, t=T, dbg=False):
    """Build + compile the Bass module for one core's shard."""
    from contextlib import ExitStack

    nc = bacc.Bacc(
        "TRN2", target_bir_lowering=False, debug=False, num_devices=N_CORES
    )
    x_ap = nc.dram_tensor("x", [bpc, t, C], FP32, kind="ExternalInput").ap()
    out_ap = nc.dram_tensor("out", [bpc, t, C], FP32, kind="ExternalOutput").ap()
    with tile.TileContext(nc) as tc:
        with ExitStack() as ctx:
            _body(ctx, tc, out_ap, x_ap, r, bpc, t, dbg=dbg)
    nc.compile()
    return nc


def kernel(x, r_sigma):
    global LAST_RESULTS
    x = np.ascontiguousarray(np.asarray(x, dtype=np.float32))
    r = float(np.asarray(r_sigma).reshape(-1)[0])
    assert x.shape == (B, T, C), x.shape

    nc = build(r)
    in_maps = [
        {"x": np.ascontiguousarray(x[i * BPC : (i + 1) * BPC])}
        for i in range(N_CORES)
    ]
    trace = bool(int(os.environ.get("KERNEL_TRACE", "0")))
    res = run_bass_kernel_spmd(
        nc, in_maps, core_ids=list(range(N_CORES)), trace=trace
    )
    LAST_RESULTS = res
    out = np.concatenate([res.results[i]["out"] for i in range(N_CORES)], axis=0)
    return out.astype(np.float32)
